# revision 14
# baseline (speedup 1.0000x reference)
"""Multi-head self-attention (RoPE + causal softmax) on 8 Trainium2 NeuronCores.

Sharding: head-parallel (Megatron). Core c owns heads {2c, 2c+1}:
  - Wq/Wk/Wv column-split -> each core projects its 128 features for all
    B*S = 4096 tokens in transposed layout [feat, t] (contraction on SBUF
    partitions). All matmuls run in bf16 (1 cycle/row on the PE vs 2 for
    fp32r) with fp32 PSUM accumulation.
  - RoPE via a partition-swap permutation matmul + DVE elementwise.
  - Attention per (batch, head) in scores-transposed layout [kp, q].
    Causal-band score tiles are column-trimmed to the live q-range and
    packed into shared [P,2,512] PSUM tiles so one exp instruction covers
    several k-tiles. Score->exp->attnV is software-pipelined with a lag of
    two units so the PE never waits on the ACT engine's exp. Softmax
    denominator comes from a ones-column appended to V in the same PSUM
    accumulation group; no max-subtraction (scores are O(1) here).
  - Wo row-split -> per-core partial y in fp16; host sums the 8 partials.
    The Wo matmuls are interleaved into the (b, hl=1) attention stream as
    soon as each 512-token aot block is complete, filling PE idle slots.
"""

from collections import deque

import numpy as np

B = 2
S = 2048
D = 1024
H = 16
HD = 64
T = B * S  # 4096
P = 128
N_CORES = 8
KT = D // P  # 8 k-tiles for the projections
N_CH = T // 512  # 8 projection chunks of 512 tokens
QC_W = 512  # attention q-chunk width
N_QC = S // QC_W  # 4 q-chunks per (batch, head)
NEG = -1.0e9

_CACHE = {}


def _build():
    import concourse.bass as bass
    import concourse.mybir as mybir
    from concourse import bacc
    from concourse.bass import ts
    from concourse.tile import TileContext

    F32 = mybir.dt.float32
    F16 = mybir.dt.float16
    BF16 = mybir.dt.bfloat16
    EXP = mybir.ActivationFunctionType.Exp

    nc = bacc.Bacc("TRN2", target_bir_lowering=False, debug=False,
                   num_devices=N_CORES)

    xt = nc.dram_tensor("xt", [D, T], BF16, kind="ExternalInput")
    wq = nc.dram_tensor("wq", [D, P], BF16, kind="ExternalInput")
    wk = nc.dram_tensor("wk", [D, P], BF16, kind="ExternalInput")
    wv = nc.dram_tensor("wv", [D, P], BF16, kind="ExternalInput")
    wo = nc.dram_tensor("wo", [P, D], BF16, kind="ExternalInput")
    cos = nc.dram_tensor("cos", [P, S], BF16, kind="ExternalInput")
    sin = nc.dram_tensor("sin", [P, S], BF16, kind="ExternalInput")
    perm = nc.dram_tensor("perm", [P, P], BF16, kind="ExternalInput")
    ident = nc.dram_tensor("ident", [P, P], BF16, kind="ExternalInput")
    cmask = nc.dram_tensor("cmask", [P, P], BF16, kind="ExternalInput")
    y = nc.dram_tensor("y", [T, D], F16, kind="ExternalOutput")

    xt_r = xt[:, :].rearrange("(ko ki) t -> ki ko t", ki=P)

    with TileContext(nc) as tc:
        with (
            tc.tile_pool(name="consts", bufs=1) as consts,
            tc.tile_pool(name="xtp", bufs=2) as xtp,
            tc.tile_pool(name="work", bufs=2) as work,
            tc.tile_pool(name="expp", bufs=10) as expp,
            tc.tile_pool(name="outp", bufs=4) as outp,
        ):
            # ---- resident tiles ----
            wq_sb = consts.tile([P, KT, P], BF16, tag="wq")
            wk_sb = consts.tile([P, KT, P], BF16, tag="wk")
            wv_sb = consts.tile([P, KT, P], BF16, tag="wv")
            wo_sb = consts.tile([P, D], BF16, tag="wo")
            cos_sb = consts.tile([P, S], BF16, tag="cos")
            sin_sb = consts.tile([P, S], BF16, tag="sin")
            perm_sb = consts.tile([P, P], BF16, tag="perm")
            id_sb = consts.tile([P, P], BF16, tag="ident")
            cm_sb = consts.tile([P, P], BF16, tag="cmask")
            rotq = consts.tile([P, T], BF16, tag="rotq")
            rotk = consts.tile([P, T], BF16, tag="rotk")
            # V in natural [kp, d] layout: [kp_part, kp_tile, head, 64 + 1 one]
            vall = consts.tile([P, T // P, 2, HD + 1], BF16, tag="vall")
            ones_row = consts.tile([1, HD], BF16, tag="ones_row")
            aot = consts.tile([P, T], BF16, tag="aot")  # attn out (transposed)

            # weights + rope tables first (stage-1 critical path); wo/cmask/
            # ones are not needed until stage 2 and are emitted after the
            # first xt chunks so they don't delay the first matmuls.
            nc.sync.dma_start(wq_sb[:], wq[:, :].rearrange(
                "(ko ki) f -> ki ko f", ki=P))
            nc.sync.dma_start(wk_sb[:], wk[:, :].rearrange(
                "(ko ki) f -> ki ko f", ki=P))
            nc.sync.dma_start(wv_sb[:], wv[:, :].rearrange(
                "(ko ki) f -> ki ko f", ki=P))
            nc.sync.dma_start(perm_sb[:], perm[:, :])
            nc.sync.dma_start(id_sb[:], ident[:, :])
            nc.sync.dma_start(cos_sb[:], cos[:, :])
            nc.sync.dma_start(sin_sb[:], sin[:, :])
            nc.gpsimd.memset(ones_row[:], 1.0)
            nc.gpsimd.memset(vall[:, :, :, HD], 1.0)

            # ---- stage 1: projections + rope + V transpose ----
            stage1 = tc.tile_pool(name="pproj", bufs=1, space="PSUM")
            pproj = stage1.__enter__()
            stage1b = tc.tile_pool(name="pswp", bufs=2, space="PSUM")
            pswp = stage1b.__enter__()
            stage1c = tc.tile_pool(name="ptrp", bufs=2, space="PSUM")
            ptrp = stage1c.__enter__()
            for ch in range(N_CH):
                xt_t = xtp.tile([P, KT, 512], BF16, tag="xt")
                nc.sync.dma_start(xt_t[:, 0:KT // 2, :],
                                  xt_r[:, 0:KT // 2, ts(ch, 512)])
                nc.sync.dma_start(xt_t[:, KT // 2:KT, :],
                                  xt_r[:, KT // 2:KT, ts(ch, 512)])
                if ch == 1:
                    # stage-2 constants, off the startup critical path
                    nc.sync.dma_start(wo_sb[:], wo[:, :])
                    nc.sync.dma_start(cm_sb[:], cmask[:, :])

                # pv first: its consumer chain (ACT copy -> PE transpose)
                # overlaps the pq/pk matmuls.
                pv = pproj.tile([P, 512], F32, tag="pv")
                pq = pproj.tile([P, 512], F32, tag="pq")
                pk = pproj.tile([P, 512], F32, tag="pk")
                for k in range(KT):
                    st, sp = (k == 0), (k == KT - 1)
                    nc.tensor.matmul(pv[:], wv_sb[:, k, :], xt_t[:, k, :],
                                     start=st, stop=sp)
                vc_t = work.tile([P, 512], BF16, tag="vchunk")
                nc.scalar.copy(vc_t[:], pv[:])
                for k in range(KT):
                    st, sp = (k == 0), (k == KT - 1)
                    nc.tensor.matmul(pq[:], wq_sb[:, k, :], xt_t[:, k, :],
                                     start=st, stop=sp)
                qc_t = work.tile([P, 512], BF16, tag="qchunk")
                nc.scalar.copy(qc_t[:], pq[:])
                for k in range(KT):
                    st, sp = (k == 0), (k == KT - 1)
                    nc.tensor.matmul(pk[:], wk_sb[:, k, :], xt_t[:, k, :],
                                     start=st, stop=sp)
                kc_t = work.tile([P, 512], BF16, tag="kchunk")
                nc.scalar.copy(kc_t[:], pk[:])

                # V: transpose [feat, t] -> natural [t, feat] via PE
                for sub in range(4):
                    ptr_t = ptrp.tile([P, P], BF16, tag="ptr")
                    nc.tensor.transpose(ptr_t[:], vc_t[:, ts(sub, P)],
                                        id_sb[:])
                    nc.vector.tensor_copy(
                        vall[:, ch * 4 + sub, :, 0:HD],
                        ptr_t[:].rearrange("p (h d) -> p h d", h=2))

                # RoPE: rot = proj * cos + swap(proj) * sin_signed
                s_sl = ts(ch % (S // 512), 512)
                for src_ps, src_sb, dst in ((pq, qc_t, rotq),
                                            (pk, kc_t, rotk)):
                    psw = pswp.tile([P, 512], F32, tag="psw")
                    nc.tensor.matmul(psw[:], perm_sb[:], src_sb[:],
                                     start=True, stop=True)
                    t1 = work.tile([P, 512], F32, tag="ropet1")
                    t2 = work.tile([P, 512], F32, tag="ropet2")
                    nc.vector.tensor_mul(t1[:], src_ps[:], cos_sb[:, s_sl])
                    nc.vector.tensor_mul(t2[:], psw[:], sin_sb[:, s_sl])
                    nc.vector.tensor_add(dst[:, ts(ch, 512)], t1[:], t2[:])

            stage1c.__exit__(None, None, None)
            stage1b.__exit__(None, None, None)
            stage1.__exit__(None, None, None)

            # ---- stage 2: attention, software-pipelined ----
            # PSUM: pss2 ring [P,2,512] bufs=3 (6 banks, shared by score
            # pairs, the denominator broadcast and the Wo py tiles) +
            # ps_o accumulators bufs=2 (2 banks).
            stage2 = tc.tile_pool(name="pss2", bufs=3, space="PSUM")
            pssp = stage2.__enter__()
            stage2b = tc.tile_pool(name="pso", bufs=2, space="PSUM")
            psop = stage2b.__enter__()

            ycopy_flip = [0]
            wo_queue = deque()  # (b, mi) token tiles ready for Wo
            norm_queue = deque()  # deferred normalize closures

            def emit_wo_one():
                b, mi = wo_queue.popleft()
                m = b * (S // P) + mi
                py2 = pssp.tile([P, 2, 512], F32, tag="pss")
                for oc in range(2):
                    nc.tensor.matmul(py2[:, oc, :], aot[:, ts(m, P)],
                                     wo_sb[:, ts(oc, 512)],
                                     start=True, stop=True)
                    y_sb = outp.tile([P, 512], F16, tag="ysb")
                    ycopy_flip[0] ^= 1
                    if ycopy_flip[0]:
                        nc.vector.tensor_copy(y_sb[:], py2[:, oc, :])
                    else:
                        nc.scalar.copy(y_sb[:], py2[:, oc, :])
                    nc.sync.dma_start(y[ts(m, P), ts(oc, 512)], y_sb[:])

            for b in range(B):
                for hl in range(2):
                    pr = slice(HD * hl, HD * hl + HD)
                    t0 = b * S

                    # Unit = score matmuls (+PE-side mask matmuls) + one exp.
                    # attnV segs lag 2 units; normalize lags 1 more unit; Wo
                    # row tiles drain from a global queue, one per unit.
                    units = []  # (qc, kind, p2)
                    for qc in range(N_QC):
                        for p2 in range(2 * qc):
                            units.append((qc, "F", p2))
                        units.append((qc, "A", None))
                        units.append((qc, "B", None))

                    qc_state = {}  # qc -> [ps_o, seg_idx, nseg]

                    def get_qc(qc):
                        if qc not in qc_state:
                            ps_o = psop.tile([P, QC_W], F32, tag="pso")
                            qc_state[qc] = [ps_o, 0, 4 * qc + 4]
                        return qc_state[qc]

                    def emit_unit(u):
                        qc, kind, p2 = u
                        q0 = t0 + QC_W * qc
                        segs = []  # (e, h2, c0, c1, qoff, t)
                        ps2 = pssp.tile([P, 2, 512], F32, tag="pss")
                        e2 = expp.tile([P, 2, 512], BF16, tag="expT")
                        if kind == "F":
                            for h2 in range(2):
                                t = 2 * p2 + h2
                                nc.tensor.matmul(
                                    ps2[:, h2, :],
                                    rotk[pr, t0 + P * t: t0 + P * (t + 1)],
                                    rotq[pr, q0:q0 + 512],
                                    start=True, stop=True)
                                segs.append((e2, h2, 0, 512, 0, t))
                            nc.scalar.activation(e2[:], ps2[:], EXP,
                                                 scale=0.125)
                        elif kind == "A":
                            for td, h2, c0, c1, qoff in (
                                    (0, 0, 0, 512, 0),
                                    (1, 1, 0, 384, 128),
                                    (3, 1, 384, 512, 384)):
                                t = 4 * qc + td
                                nc.tensor.matmul(
                                    ps2[:, h2, c0:c1],
                                    rotk[pr, t0 + P * t: t0 + P * (t + 1)],
                                    rotq[pr, q0 + qoff:q0 + 512],
                                    start=True, stop=False)
                                # triangle mask via PE: out += cmT.T @ I
                                mc = c0 if td != 1 else 0
                                nc.tensor.matmul(
                                    ps2[:, h2, mc:mc + P], cm_sb[:],
                                    id_sb[:], start=False, stop=True,
                                    skip_group_check=True)
                                segs.append((e2, h2, c0, c1, qoff, t))
                            nc.scalar.activation(e2[:], ps2[:], EXP,
                                                 scale=0.125)
                        else:  # "B"
                            t = 4 * qc + 2
                            nc.tensor.matmul(
                                ps2[:, 0, 0:256],
                                rotk[pr, t0 + P * t: t0 + P * (t + 1)],
                                rotq[pr, q0 + 256:q0 + 512],
                                start=True, stop=False)
                            nc.tensor.matmul(
                                ps2[:, 0, 0:P], cm_sb[:], id_sb[:],
                                start=False, stop=True,
                                skip_group_check=True)
                            nc.scalar.activation(e2[:, 0, 0:256],
                                                 ps2[:, 0, 0:256], EXP,
                                                 scale=0.125)
                            segs.append((e2, 0, 0, 256, 256, t))
                        segs.sort(key=lambda s: s[5])
                        return segs

                    def emit_att(u, segs):
                        qc, kind, _ = u
                        st = get_qc(qc)
                        ps_o, _, nseg = st
                        q0 = t0 + QC_W * qc
                        for (e2, h2, c0, c1, qoff, t) in segs:
                            i = st[1]
                            st[1] += 1
                            w = c1 - c0
                            nc.tensor.matmul(
                                ps_o[0:HD + 1, qoff:qoff + w],
                                vall[:, b * (S // P) + t, hl, :],
                                e2[:, h2, c0:c1],
                                start=(i == 0), stop=(i == nseg - 1),
                                skip_group_check=True)
                        if st[1] == nseg:
                            # denominator row -> SBUF now (DVE); the rest of
                            # the normalize chain is deferred one unit so the
                            # PE never waits on it.
                            dn = work.tile([1, QC_W], BF16, tag="denr")
                            nc.vector.tensor_copy(dn[:], ps_o[HD:HD + 1, :])
                            del qc_state[qc]

                            def norm(qc=qc, ps_o=ps_o, dn=dn, b=b, hl=hl,
                                     t0=t0, pr=pr):
                                pbt = pssp.tile([P, 2, 512], F32, tag="pss")
                                nc.tensor.matmul(pbt[0:HD, 0, :],
                                                 ones_row[:], dn[:],
                                                 start=True, stop=True)
                                rb_sb = work.tile([HD, QC_W], F32,
                                                  tag="rbsb")
                                nc.vector.reciprocal_approx_fast(
                                    rb_sb[:], pbt[0:HD, 0, :])
                                q0 = t0 + QC_W * qc
                                nc.vector.tensor_mul(
                                    aot[pr, q0:q0 + QC_W],
                                    ps_o[0:HD, :], rb_sb[:])
                                if hl == 1:
                                    for mi in range(4 * qc, 4 * qc + 4):
                                        wo_queue.append((b, mi))

                            norm_queue.append([uidx[0] + 2, norm])

                    pending = deque()
                    uidx = [0]
                    for u in units:
                        uidx[0] += 1
                        segs = emit_unit(u)
                        while norm_queue and norm_queue[0][0] <= uidx[0]:
                            norm_queue.popleft()[1]()
                        pending.append((u, segs))
                        if len(pending) > 2:
                            emit_att(*pending.popleft())
                        # Wo drain paces PE filler: every unit in hl=0
                        # streams (no other PE slack), every 2nd in hl=1.
                        if wo_queue and (hl == 0 or uidx[0] % 2 == 0):
                            emit_wo_one()
                    while pending:
                        emit_att(*pending.popleft())
                        uidx[0] += 1
                        while norm_queue and norm_queue[0][0] <= uidx[0]:
                            norm_queue.popleft()[1]()
                        if wo_queue:
                            emit_wo_one()
                    while norm_queue:
                        norm_queue.popleft()[1]()
            while wo_queue:
                emit_wo_one()

            stage2b.__exit__(None, None, None)
            stage2.__exit__(None, None, None)

    nc.compile()
    return nc


def _host_prep(x, token_positions, Wq, Wk, Wv, Wo, rope_sin, rope_cos):
    import ml_dtypes
    bf16 = ml_dtypes.bfloat16

    x = np.asarray(x, dtype=np.float32)
    Wq = np.asarray(Wq, dtype=np.float32)
    Wk = np.asarray(Wk, dtype=np.float32)
    Wv = np.asarray(Wv, dtype=np.float32)
    Wo = np.asarray(Wo, dtype=np.float32)
    pos = np.asarray(token_positions).astype(np.int64)
    sin_g = np.asarray(rope_sin, dtype=np.float32)[pos]  # [S, 32]
    cos_g = np.asarray(rope_cos, dtype=np.float32)[pos]

    xt = np.ascontiguousarray(x.reshape(T, D).T).astype(bf16)  # [D, T]

    j = np.arange(P) % 32
    cosE = np.ascontiguousarray(cos_g.T[j, :])  # [128, S]
    sgn = np.where((np.arange(P) % HD) < 32, -1.0, 1.0).astype(np.float32)
    sinS = np.ascontiguousarray(sgn[:, None] * sin_g.T[j, :])

    p_idx = np.arange(P)
    swap = (p_idx // HD) * HD + ((p_idx % HD) + 32) % HD
    perm = np.zeros((P, P), dtype=np.float32)
    perm[swap, p_idx] = 1.0
    ident = np.eye(P, dtype=np.float32)

    # triangle mask as matmul stationary: out[p, j] += cmask[j, p] with an
    # identity moving operand; masks iff j < p (q-local j, k-local p)
    jj = np.arange(P)[:, None]
    pp = np.arange(P)[None, :]
    cmask = np.where(jj < pp, NEG, 0.0).astype(np.float32)

    in_maps = []
    for c in range(N_CORES):
        feats = []
        for hl in range(2):
            h = 2 * c + hl
            base = h * HD
            feats.extend(base + 2 * np.arange(32))      # x1 (even d)
            feats.extend(base + 2 * np.arange(32) + 1)  # x2 (odd d)
        feats = np.array(feats)
        nat = np.arange(2 * c * HD, (2 * c + 2) * HD)
        in_maps.append({
            "xt": xt,
            "wq": np.ascontiguousarray(Wq[feats, :].T).astype(bf16),
            "wk": np.ascontiguousarray(Wk[feats, :].T).astype(bf16),
            "wv": np.ascontiguousarray(Wv[nat, :].T).astype(bf16),
            "wo": np.ascontiguousarray(Wo[:, nat].T).astype(bf16),
            "cos": cosE.astype(bf16), "sin": sinS.astype(bf16),
            "perm": perm.astype(bf16), "ident": ident.astype(bf16),
            "cmask": cmask.astype(bf16),
        })
    return in_maps


def run(trace=False, **inputs):
    from concourse.bass_utils import run_bass_kernel_spmd

    if "nc" not in _CACHE:
        _CACHE["nc"] = _build()
    nc = _CACHE["nc"]
    in_maps = _host_prep(**inputs)
    res = run_bass_kernel_spmd(nc, in_maps, core_ids=list(range(N_CORES)),
                               trace=trace)
    out = np.zeros((T, D), dtype=np.float32)
    for c in range(N_CORES):
        out += res.results[c]["y"].astype(np.float32)
    return out.reshape(B, S, D), res


def kernel(**inputs) -> np.ndarray:
    out, _ = run(trace=False, **inputs)
    return out


# revision 16
# speedup vs baseline: 1.1359x; 1.1359x over previous
"""Multi-head self-attention (RoPE + causal softmax) on 8 Trainium2 NeuronCores.

Sharding: head-parallel (Megatron). Core c owns heads {2c, 2c+1}:
  - Wq/Wk/Wv column-split -> each core projects its 128 features for all
    B*S = 4096 tokens in transposed layout [feat, t] (contraction on SBUF
    partitions). All matmuls run in bf16 (1 cycle/row on the PE vs 2 for
    fp32r) with fp32 PSUM accumulation.
  - RoPE via a partition-swap permutation matmul + DVE elementwise.
  - Attention per (batch, head) in scores-transposed layout [kp, q].
    Causal-band score tiles are column-trimmed to the live q-range and
    packed into shared [P,2,512] PSUM tiles so one exp instruction covers
    several k-tiles. Score->exp->attnV is software-pipelined with a lag of
    two units so the PE never waits on the ACT engine's exp. Softmax
    denominator comes from a ones-column appended to V in the same PSUM
    accumulation group; no max-subtraction (scores are O(1) here).
  - Wo row-split -> per-core partial y in fp16; host sums the 8 partials.
    The Wo matmuls are interleaved into the (b, hl=1) attention stream as
    soon as each 512-token aot block is complete, filling PE idle slots.
"""

from collections import deque

import numpy as np

B = 2
S = 2048
D = 1024
H = 16
HD = 64
T = B * S  # 4096
P = 128
N_CORES = 8
KT = D // P  # 8 k-tiles for the projections
N_CH = T // 512  # 8 projection chunks of 512 tokens
QC_W = 512  # attention q-chunk width
N_QC = S // QC_W  # 4 q-chunks per (batch, head)
NEG = -1.0e9

_CACHE = {}


def _build():
    import concourse.bass as bass
    import concourse.mybir as mybir
    from concourse import bacc
    from concourse.bass import ts
    from concourse.tile import TileContext

    F32 = mybir.dt.float32
    F16 = mybir.dt.float16
    BF16 = mybir.dt.bfloat16
    I16 = mybir.dt.int16
    EXP = mybir.ActivationFunctionType.Exp
    MULT = mybir.AluOpType.mult
    ADD = mybir.AluOpType.add
    # Schraudolph: bf16 bits of exp(s*0.125) ~= s*SCH_A + SCH_B as int16.
    # Mean-centered (C=-7.4); |s|<=~170 keeps the bits far from saturation.
    SCH_A = 0.125 * 1.4426950408889634 * 128.0
    SCH_B = 127.0 * 128.0 - 7.4

    nc = bacc.Bacc("TRN2", target_bir_lowering=False, debug=False,
                   num_devices=N_CORES)

    xt = nc.dram_tensor("xt", [D, T], BF16, kind="ExternalInput")
    wq = nc.dram_tensor("wq", [D, P], BF16, kind="ExternalInput")
    wk = nc.dram_tensor("wk", [D, P], BF16, kind="ExternalInput")
    wv = nc.dram_tensor("wv", [D, P], BF16, kind="ExternalInput")
    wo = nc.dram_tensor("wo", [P, D], BF16, kind="ExternalInput")
    cos = nc.dram_tensor("cos", [P, S], BF16, kind="ExternalInput")
    sin = nc.dram_tensor("sin", [P, S], BF16, kind="ExternalInput")
    perm = nc.dram_tensor("perm", [P, P], BF16, kind="ExternalInput")
    ident = nc.dram_tensor("ident", [P, P], BF16, kind="ExternalInput")
    cmask = nc.dram_tensor("cmask", [P, P], BF16, kind="ExternalInput")
    y = nc.dram_tensor("y", [T, D], F16, kind="ExternalOutput")

    xt_r = xt[:, :].rearrange("(ko ki) t -> ki ko t", ki=P)

    with TileContext(nc) as tc:
        with (
            tc.tile_pool(name="consts", bufs=1) as consts,
            tc.tile_pool(name="xtp", bufs=2) as xtp,
            tc.tile_pool(name="work", bufs=2) as work,
            tc.tile_pool(name="expp", bufs=10) as expp,
            tc.tile_pool(name="outp", bufs=4) as outp,
        ):
            # ---- resident tiles ----
            wq_sb = consts.tile([P, KT, P], BF16, tag="wq")
            wk_sb = consts.tile([P, KT, P], BF16, tag="wk")
            wv_sb = consts.tile([P, KT, P], BF16, tag="wv")
            wo_sb = consts.tile([P, D], BF16, tag="wo")
            cos_sb = consts.tile([P, S], BF16, tag="cos")
            sin_sb = consts.tile([P, S], BF16, tag="sin")
            perm_sb = consts.tile([P, P], BF16, tag="perm")
            id_sb = consts.tile([P, P], BF16, tag="ident")
            cm_sb = consts.tile([P, P], BF16, tag="cmask")
            rotq = consts.tile([P, T], BF16, tag="rotq")
            rotk = consts.tile([P, T], BF16, tag="rotk")
            # V in natural [kp, d] layout: [kp_part, kp_tile, head, 64 + 1 one]
            vall = consts.tile([P, T // P, 2, HD + 1], BF16, tag="vall")
            ones_row = consts.tile([1, HD], BF16, tag="ones_row")
            aot = consts.tile([P, T], BF16, tag="aot")  # attn out (transposed)

            # weights + rope tables first (stage-1 critical path); wo/cmask/
            # ones are not needed until stage 2 and are emitted after the
            # first xt chunks so they don't delay the first matmuls.
            nc.sync.dma_start(wq_sb[:], wq[:, :].rearrange(
                "(ko ki) f -> ki ko f", ki=P))
            nc.sync.dma_start(wk_sb[:], wk[:, :].rearrange(
                "(ko ki) f -> ki ko f", ki=P))
            nc.sync.dma_start(wv_sb[:], wv[:, :].rearrange(
                "(ko ki) f -> ki ko f", ki=P))
            nc.sync.dma_start(perm_sb[:], perm[:, :])
            nc.sync.dma_start(id_sb[:], ident[:, :])
            nc.sync.dma_start(cos_sb[:], cos[:, :])
            nc.sync.dma_start(sin_sb[:], sin[:, :])
            nc.gpsimd.memset(ones_row[:], 1.0)
            nc.gpsimd.memset(vall[:, :, :, HD], 1.0)

            # ---- stage 1: projections + rope + V transpose ----
            stage1 = tc.tile_pool(name="pproj", bufs=1, space="PSUM")
            pproj = stage1.__enter__()
            stage1b = tc.tile_pool(name="pswp", bufs=2, space="PSUM")
            pswp = stage1b.__enter__()
            stage1c = tc.tile_pool(name="ptrp", bufs=2, space="PSUM")
            ptrp = stage1c.__enter__()
            for ch in range(N_CH):
                xt_t = xtp.tile([P, KT, 512], BF16, tag="xt")
                nc.sync.dma_start(xt_t[:, 0:KT // 2, :],
                                  xt_r[:, 0:KT // 2, ts(ch, 512)])
                nc.sync.dma_start(xt_t[:, KT // 2:KT, :],
                                  xt_r[:, KT // 2:KT, ts(ch, 512)])
                if ch == 1:
                    # stage-2 constants, off the startup critical path
                    nc.sync.dma_start(wo_sb[:], wo[:, :])
                    nc.sync.dma_start(cm_sb[:], cmask[:, :])

                # pv first: its consumer chain (ACT copy -> PE transpose)
                # overlaps the pq/pk matmuls.
                pv = pproj.tile([P, 512], F32, tag="pv")
                pq = pproj.tile([P, 512], F32, tag="pq")
                pk = pproj.tile([P, 512], F32, tag="pk")
                for k in range(KT):
                    st, sp = (k == 0), (k == KT - 1)
                    nc.tensor.matmul(pv[:], wv_sb[:, k, :], xt_t[:, k, :],
                                     start=st, stop=sp)
                vc_t = work.tile([P, 512], BF16, tag="vchunk")
                nc.scalar.copy(vc_t[:], pv[:])
                for k in range(KT):
                    st, sp = (k == 0), (k == KT - 1)
                    nc.tensor.matmul(pq[:], wq_sb[:, k, :], xt_t[:, k, :],
                                     start=st, stop=sp)
                qc_t = work.tile([P, 512], BF16, tag="qchunk")
                nc.scalar.copy(qc_t[:], pq[:])
                for k in range(KT):
                    st, sp = (k == 0), (k == KT - 1)
                    nc.tensor.matmul(pk[:], wk_sb[:, k, :], xt_t[:, k, :],
                                     start=st, stop=sp)
                kc_t = work.tile([P, 512], BF16, tag="kchunk")
                nc.scalar.copy(kc_t[:], pk[:])

                # V: transpose [feat, t] -> natural [t, feat] via PE
                for sub in range(4):
                    ptr_t = ptrp.tile([P, P], BF16, tag="ptr")
                    nc.tensor.transpose(ptr_t[:], vc_t[:, ts(sub, P)],
                                        id_sb[:])
                    nc.vector.tensor_copy(
                        vall[:, ch * 4 + sub, :, 0:HD],
                        ptr_t[:].rearrange("p (h d) -> p h d", h=2))

                # RoPE: rot = proj * cos + swap(proj) * sin_signed
                s_sl = ts(ch % (S // 512), 512)
                for src_ps, src_sb, dst in ((pq, qc_t, rotq),
                                            (pk, kc_t, rotk)):
                    psw = pswp.tile([P, 512], F32, tag="psw")
                    nc.tensor.matmul(psw[:], perm_sb[:], src_sb[:],
                                     start=True, stop=True)
                    t1 = work.tile([P, 512], F32, tag="ropet1")
                    t2 = work.tile([P, 512], F32, tag="ropet2")
                    nc.vector.tensor_mul(t1[:], src_ps[:], cos_sb[:, s_sl])
                    nc.vector.tensor_mul(t2[:], psw[:], sin_sb[:, s_sl])
                    nc.vector.tensor_add(dst[:, ts(ch, 512)], t1[:], t2[:])

            stage1c.__exit__(None, None, None)
            stage1b.__exit__(None, None, None)
            stage1.__exit__(None, None, None)

            # ---- stage 2: attention, software-pipelined ----
            # PSUM: pss2 ring [P,2,512] bufs=3 (6 banks, shared by score
            # pairs, the denominator broadcast and the Wo py tiles) +
            # ps_o accumulators bufs=2 (2 banks).
            stage2 = tc.tile_pool(name="pss2", bufs=3, space="PSUM")
            pssp = stage2.__enter__()
            stage2b = tc.tile_pool(name="pso", bufs=2, space="PSUM")
            psop = stage2b.__enter__()

            ycopy_flip = [0]
            wo_queue = deque()  # (b, mi) token tiles ready for Wo
            norm_queue = deque()  # deferred normalize closures

            def emit_wo_one():
                b, mi = wo_queue.popleft()
                m = b * (S // P) + mi
                py2 = pssp.tile([P, 2, 512], F32, tag="pss")
                for oc in range(2):
                    nc.tensor.matmul(py2[:, oc, :], aot[:, ts(m, P)],
                                     wo_sb[:, ts(oc, 512)],
                                     start=True, stop=True)
                    y_sb = outp.tile([P, 512], F16, tag="ysb")
                    ycopy_flip[0] ^= 1
                    if ycopy_flip[0]:
                        nc.vector.tensor_copy(y_sb[:], py2[:, oc, :])
                    else:
                        nc.scalar.copy(y_sb[:], py2[:, oc, :])
                    nc.sync.dma_start(y[ts(m, P), ts(oc, 512)], y_sb[:])

            for b in range(B):
                for hl in range(2):
                    pr = slice(HD * hl, HD * hl + HD)
                    t0 = b * S

                    # Unit = score matmuls (+PE-side mask matmuls) + one exp.
                    # attnV segs lag 2 units; normalize lags 1 more unit; Wo
                    # row tiles drain from a global queue, one per unit.
                    units = []  # (qc, kind, p2)
                    for qc in range(N_QC):
                        for p2 in range(2 * qc):
                            units.append((qc, "F", p2))
                        units.append((qc, "A", None))
                        units.append((qc, "B", None))

                    qc_state = {}  # qc -> [ps_o, seg_idx, nseg]

                    def get_qc(qc):
                        if qc not in qc_state:
                            ps_o = psop.tile([P, QC_W], F32, tag="pso")
                            qc_state[qc] = [ps_o, 0, 4 * qc + 4]
                        return qc_state[qc]

                    def emit_unit(u):
                        qc, kind, p2 = u
                        q0 = t0 + QC_W * qc
                        segs = []  # (e, h2, c0, c1, qoff, t)
                        ps2 = pssp.tile([P, 2, 512], F32, tag="pss")
                        if kind == "F" and p2 % 2 == 1:
                            # full tiles (no masked elements): Schraudolph
                            # exp on the DVE to offload the ACT engine.
                            ei = expp.tile([P, 2, 512], I16, tag="expI",
                                           bufs=6)
                            e2 = ei.bitcast(BF16)
                            for h2 in range(2):
                                t = 2 * p2 + h2
                                nc.tensor.matmul(
                                    ps2[:, h2, :],
                                    rotk[pr, t0 + P * t: t0 + P * (t + 1)],
                                    rotq[pr, q0:q0 + 512],
                                    start=True, stop=True)
                                segs.append((e2, h2, 0, 512, 0, t))
                            nc.vector.tensor_scalar(
                                ei[:], ps2[:], SCH_A, SCH_B, MULT, ADD)
                            segs.sort(key=lambda s: s[5])
                            return segs
                        e2 = expp.tile([P, 2, 512], BF16, tag="expT")
                        if kind == "F":
                            for h2 in range(2):
                                t = 2 * p2 + h2
                                nc.tensor.matmul(
                                    ps2[:, h2, :],
                                    rotk[pr, t0 + P * t: t0 + P * (t + 1)],
                                    rotq[pr, q0:q0 + 512],
                                    start=True, stop=True)
                                segs.append((e2, h2, 0, 512, 0, t))
                            nc.scalar.activation(e2[:], ps2[:], EXP,
                                                 scale=0.125)
                        elif kind == "A":
                            for td, h2, c0, c1, qoff in (
                                    (0, 0, 0, 512, 0),
                                    (1, 1, 0, 384, 128),
                                    (3, 1, 384, 512, 384)):
                                t = 4 * qc + td
                                nc.tensor.matmul(
                                    ps2[:, h2, c0:c1],
                                    rotk[pr, t0 + P * t: t0 + P * (t + 1)],
                                    rotq[pr, q0 + qoff:q0 + 512],
                                    start=True, stop=False)
                                # triangle mask via PE: out += cmT.T @ I
                                mc = c0 if td != 1 else 0
                                nc.tensor.matmul(
                                    ps2[:, h2, mc:mc + P], cm_sb[:],
                                    id_sb[:], start=False, stop=True,
                                    skip_group_check=True)
                                segs.append((e2, h2, c0, c1, qoff, t))
                            nc.scalar.activation(e2[:], ps2[:], EXP,
                                                 scale=0.125)
                        else:  # "B"
                            t = 4 * qc + 2
                            nc.tensor.matmul(
                                ps2[:, 0, 0:256],
                                rotk[pr, t0 + P * t: t0 + P * (t + 1)],
                                rotq[pr, q0 + 256:q0 + 512],
                                start=True, stop=False)
                            nc.tensor.matmul(
                                ps2[:, 0, 0:P], cm_sb[:], id_sb[:],
                                start=False, stop=True,
                                skip_group_check=True)
                            nc.scalar.activation(e2[:, 0, 0:256],
                                                 ps2[:, 0, 0:256], EXP,
                                                 scale=0.125)
                            segs.append((e2, 0, 0, 256, 256, t))
                        segs.sort(key=lambda s: s[5])
                        return segs

                    def emit_att(u, segs):
                        qc, kind, _ = u
                        st = get_qc(qc)
                        ps_o, _, nseg = st
                        q0 = t0 + QC_W * qc
                        for (e2, h2, c0, c1, qoff, t) in segs:
                            i = st[1]
                            st[1] += 1
                            w = c1 - c0
                            nc.tensor.matmul(
                                ps_o[0:HD + 1, qoff:qoff + w],
                                vall[:, b * (S // P) + t, hl, :],
                                e2[:, h2, c0:c1],
                                start=(i == 0), stop=(i == nseg - 1),
                                skip_group_check=True)
                        if st[1] == nseg:
                            # denominator row -> SBUF now (DVE); the rest of
                            # the normalize chain is deferred one unit so the
                            # PE never waits on it.
                            dn = work.tile([1, QC_W], BF16, tag="denr")
                            nc.vector.tensor_copy(dn[:], ps_o[HD:HD + 1, :])
                            del qc_state[qc]

                            def norm(qc=qc, ps_o=ps_o, dn=dn, b=b, hl=hl,
                                     t0=t0, pr=pr):
                                pbt = pssp.tile([P, 2, 512], F32, tag="pss")
                                nc.tensor.matmul(pbt[0:HD, 0, :],
                                                 ones_row[:], dn[:],
                                                 start=True, stop=True)
                                rb_sb = work.tile([HD, QC_W], F32,
                                                  tag="rbsb")
                                nc.vector.reciprocal_approx_fast(
                                    rb_sb[:], pbt[0:HD, 0, :])
                                q0 = t0 + QC_W * qc
                                nc.vector.tensor_mul(
                                    aot[pr, q0:q0 + QC_W],
                                    ps_o[0:HD, :], rb_sb[:])
                                if hl == 1:
                                    for mi in range(4 * qc, 4 * qc + 4):
                                        wo_queue.append((b, mi))

                            norm_queue.append([uidx[0] + 2, norm])

                    pending = deque()
                    uidx = [0]
                    for u in units:
                        uidx[0] += 1
                        segs = emit_unit(u)
                        while norm_queue and norm_queue[0][0] <= uidx[0]:
                            norm_queue.popleft()[1]()
                        pending.append((u, segs))
                        if len(pending) > 2:
                            emit_att(*pending.popleft())
                        # Wo drain paces PE filler: every unit in hl=0
                        # streams (no other PE slack), every 2nd in hl=1.
                        if wo_queue and (hl == 0 or uidx[0] % 2 == 0):
                            emit_wo_one()
                    while pending:
                        emit_att(*pending.popleft())
                        uidx[0] += 1
                        while norm_queue and norm_queue[0][0] <= uidx[0]:
                            norm_queue.popleft()[1]()
                        if wo_queue:
                            emit_wo_one()
                    while norm_queue:
                        norm_queue.popleft()[1]()
            while wo_queue:
                emit_wo_one()

            stage2b.__exit__(None, None, None)
            stage2.__exit__(None, None, None)

    nc.compile()
    return nc


def _host_prep(x, token_positions, Wq, Wk, Wv, Wo, rope_sin, rope_cos):
    import ml_dtypes
    bf16 = ml_dtypes.bfloat16

    x = np.asarray(x, dtype=np.float32)
    Wq = np.asarray(Wq, dtype=np.float32)
    Wk = np.asarray(Wk, dtype=np.float32)
    Wv = np.asarray(Wv, dtype=np.float32)
    Wo = np.asarray(Wo, dtype=np.float32)
    pos = np.asarray(token_positions).astype(np.int64)
    sin_g = np.asarray(rope_sin, dtype=np.float32)[pos]  # [S, 32]
    cos_g = np.asarray(rope_cos, dtype=np.float32)[pos]

    xt = np.ascontiguousarray(x.reshape(T, D).T).astype(bf16)  # [D, T]

    j = np.arange(P) % 32
    cosE = np.ascontiguousarray(cos_g.T[j, :])  # [128, S]
    sgn = np.where((np.arange(P) % HD) < 32, -1.0, 1.0).astype(np.float32)
    sinS = np.ascontiguousarray(sgn[:, None] * sin_g.T[j, :])

    p_idx = np.arange(P)
    swap = (p_idx // HD) * HD + ((p_idx % HD) + 32) % HD
    perm = np.zeros((P, P), dtype=np.float32)
    perm[swap, p_idx] = 1.0
    ident = np.eye(P, dtype=np.float32)

    # triangle mask as matmul stationary: out[p, j] += cmask[j, p] with an
    # identity moving operand; masks iff j < p (q-local j, k-local p)
    jj = np.arange(P)[:, None]
    pp = np.arange(P)[None, :]
    cmask = np.where(jj < pp, NEG, 0.0).astype(np.float32)

    in_maps = []
    for c in range(N_CORES):
        feats = []
        for hl in range(2):
            h = 2 * c + hl
            base = h * HD
            feats.extend(base + 2 * np.arange(32))      # x1 (even d)
            feats.extend(base + 2 * np.arange(32) + 1)  # x2 (odd d)
        feats = np.array(feats)
        nat = np.arange(2 * c * HD, (2 * c + 2) * HD)
        in_maps.append({
            "xt": xt,
            "wq": np.ascontiguousarray(Wq[feats, :].T).astype(bf16),
            "wk": np.ascontiguousarray(Wk[feats, :].T).astype(bf16),
            "wv": np.ascontiguousarray(Wv[nat, :].T).astype(bf16),
            "wo": np.ascontiguousarray(Wo[:, nat].T).astype(bf16),
            "cos": cosE.astype(bf16), "sin": sinS.astype(bf16),
            "perm": perm.astype(bf16), "ident": ident.astype(bf16),
            "cmask": cmask.astype(bf16),
        })
    return in_maps


def run(trace=False, **inputs):
    from concourse.bass_utils import run_bass_kernel_spmd

    if "nc" not in _CACHE:
        _CACHE["nc"] = _build()
    nc = _CACHE["nc"]
    in_maps = _host_prep(**inputs)
    res = run_bass_kernel_spmd(nc, in_maps, core_ids=list(range(N_CORES)),
                               trace=trace)
    out = np.zeros((T, D), dtype=np.float32)
    for c in range(N_CORES):
        out += res.results[c]["y"].astype(np.float32)
    return out.reshape(B, S, D), res


def kernel(**inputs) -> np.ndarray:
    out, _ = run(trace=False, **inputs)
    return out
